# revision 30
# baseline (speedup 1.0000x reference)
"""Distributed Trainium2 kernel for nn_AdjLayer (conv3x3 -> softmax -> outer(colsum)).

Raw-Bacc implementation (no TileContext) with a manual semaphore pipeline.

  - Host: im2col the tiny input (48400 x 28, incl. ones column for bias) in
    bf16, shard 6050 pixels per core, pad to 6144, pack as [128, 2204] with
    3 pixel-groups at partition offsets 0/32/64 (28 im2col rows each) and
    the weights in the first 156 columns.
  - Device (SPMD x8), 16 superblocks of 3 x 128-pixel tiles:
      PE:   3 bf16 matmuls (K=28) per block -> PSUM bank b%8 [128, 468]
      ACT:  one wide Exp (bf16 out) per block into a 16-slot exp buffer;
            normalize of tile u2 for blocks 0-7 (Relu x scale == multiply)
      DVE:  4-block batched row-sum reduces (bf16 in, f32 out); batched
            reciprocals; 4D-batched broadcast multiplies (bf16 out) for
            tiles u0,u1 of all blocks and u2 of blocks 8-15
      SYNC: S streamed out bf16 in 12 chunked DMAs (4 tiles each)
  - Host: gather bf16 S shards -> f32, c = S.sum(0), new_adj = outer(c, c).
"""

import sys
from contextlib import ExitStack

import numpy as np

for _p in ("/opt/trn_rl_repo",):
    if _p not in sys.path:
        sys.path.insert(0, _p)

N_F = 156          # filters
N_PIX = 48400      # 220*220 output pixels
N_CORES = 8
PER_CORE = N_PIX // N_CORES   # 6050
K = 28             # 27 conv taps + 1 ones-row (bias)
GROUPS = 3         # pixel groups at partition offsets 0/32/64
TILE = 128         # pixels per matmul tile
TILES_PER_G = 16
G_PIX = TILES_PER_G * TILE        # 2048 pixels per group
PIX_PAD = GROUPS * G_PIX          # 6144 padded pixels per core
N_TILES = GROUPS * TILES_PER_G    # 48
SB = 3                            # tiles per superblock
N_SB = N_TILES // SB              # 16
RB = 4                            # superblocks per reduce/reciprocal batch
N_RB = N_SB // RB                 # 4
CHUNK = 4                         # tiles per output DMA (12 DMAs)
N_CHUNKS = N_TILES // CHUNK
XW_COLS = N_F + G_PIX             # weights cols [0:156], im2col [156:2204]
SPLIT = N_F + 12 * TILE           # input dma1 covers W + tiles j<12
N_BANK = 8                        # psum banks in flight
N_ACT_U2 = 8                      # blocks whose u2 tile is normalized on ACT

_GRAPH = None


def _build_graph():
    from concourse import bacc
    from concourse import mybir
    from concourse.ap import AP as RawAP

    f32 = mybir.dt.float32
    bf16 = mybir.dt.bfloat16
    nc = bacc.Bacc(None)

    xw_ext = nc.declare_dram_parameter("xw", [128, XW_COLS], bf16, isOutput=False)
    # [48, 128, 156] is byte-identical to [6144, 156] (tile-major rows)
    s_ext = nc.declare_dram_parameter("s", [N_TILES, TILE, N_F], bf16, isOutput=True)

    Exp = mybir.ActivationFunctionType.Exp
    Relu = mybir.ActivationFunctionType.Relu
    X = mybir.AxisListType.X

    W_SLOT = SB * N_F  # 468 columns per exp-buffer slot

    # ---- completion bookkeeping -------------------------------------
    # DVE s_mul increments, in emission order, with the sst tiles each
    # op stages.  ACT s_mula increments stage tile 3*bb+2 for bb < 8.
    mul_ops = []  # list of (q, kind, tiles)
    for q in range(N_RB):
        b0 = RB * q
        mul_ops.append((q, "u01", [3 * b0, 3 * b0 + 1, 3 * b0 + 3, 3 * b0 + 4]))
        mul_ops.append(
            (q, "u01b", [3 * b0 + 6, 3 * b0 + 7, 3 * b0 + 9, 3 * b0 + 10])
        )
        if b0 >= N_ACT_U2:
            mul_ops.append((q, "u2", [3 * b0 + 2, 3 * b0 + 5]))
            mul_ops.append((q, "u2b", [3 * b0 + 8, 3 * b0 + 11]))
    tile_mul_val = {}
    for i, (_, _, tiles) in enumerate(mul_ops):
        for t in tiles:
            tile_mul_val[t] = i + 1
    tile_mula_val = {3 * bb + 2: bb + 1 for bb in range(N_ACT_U2)}
    muls_before_batch = [0] * N_RB
    for i, (q, _, _) in enumerate(mul_ops):
        for qq in range(q + 1, N_RB):
            muls_before_batch[qq] = i + 1
    MUL_THR = []
    MULA_THR = []
    for c in range(N_CHUNKS):
        tiles = range(CHUNK * c, CHUNK * (c + 1))
        MUL_THR.append(max((tile_mul_val.get(t, 0) for t in tiles), default=0))
        MULA_THR.append(max((tile_mula_val.get(t, 0) for t in tiles), default=0))

    with ExitStack() as ctx:
        xw_sb = ctx.enter_context(nc.sbuf_tensor("xw_sb", [128, XW_COLS], bf16))
        sst = ctx.enter_context(
            nc.sbuf_tensor("sst", [128, N_TILES * N_F], bf16)
        )
        sums_w = ctx.enter_context(nc.sbuf_tensor("sums_w", [128, N_TILES], f32))
        recip_w = ctx.enter_context(nc.sbuf_tensor("recip_w", [128, N_TILES], f32))
        exp_buf = ctx.enter_context(
            nc.sbuf_tensor("exp_buf", [128, N_SB * W_SLOT], bf16)
        )
        psum_banks = [
            ctx.enter_context(nc.psum_tensor(f"psbank{i}", [128, W_SLOT], f32))
            for i in range(N_BANK)
        ]
        s_in = ctx.enter_context(nc.semaphore("s_in"))
        s_in2 = ctx.enter_context(nc.semaphore("s_in2"))
        s_mm = ctx.enter_context(nc.semaphore("s_mm"))
        s_exp = ctx.enter_context(nc.semaphore("s_exp"))
        s_red = ctx.enter_context(nc.semaphore("s_red"))
        s_rc = ctx.enter_context(nc.semaphore("s_rc"))
        s_mul = ctx.enter_context(nc.semaphore("s_mul"))
        s_mula = ctx.enter_context(nc.semaphore("s_mula"))
        s_out = ctx.enter_context(nc.semaphore("s_out"))

        block = ctx.enter_context(nc.Block(no_gpsimd_drain=True))

        def exp_slot(b):
            return exp_buf[:, b * W_SLOT : (b + 1) * W_SLOT]

        @block.sync
        def _(sync):
            sync.dma_start(
                out=xw_sb[:, :SPLIT], in_=xw_ext[:, :SPLIT]
            ).then_inc(s_in, 16)
            for c in range(N_CHUNKS):
                if MUL_THR[c]:
                    sync.wait_ge(s_mul, MUL_THR[c])
                if MULA_THR[c]:
                    sync.wait_ge(s_mula, MULA_THR[c])
                t0 = c * CHUNK
                dst = s_ext[t0 : t0 + CHUNK].rearrange("t p f -> p t f")
                src = sst[:, t0 * N_F : (t0 + CHUNK) * N_F].rearrange(
                    "p (t f) -> p t f", t=CHUNK
                )
                sync.dma_start(out=dst, in_=src).then_inc(s_out, 16)
            sync.wait_ge(s_out, 16 * N_CHUNKS)

        @block.tensor
        def _(tensor):
            tensor.wait_ge(s_in, 16)
            for b in range(N_SB):
                if b == 4:
                    tensor.wait_ge(s_in2, 16)
                if b >= N_BANK:
                    tensor.wait_ge(s_exp, b - N_BANK + 1)
                pb = psum_banks[b % N_BANK]
                for u in range(SB):
                    t = SB * b + u
                    g, j = divmod(t, TILES_PER_G)
                    p0 = 32 * g
                    ins = nc.tensor.matmul(
                        pb[:, u * N_F : (u + 1) * N_F],
                        lhsT=xw_sb[
                            p0 : p0 + K, N_F + j * TILE : N_F + (j + 1) * TILE
                        ],
                        rhs=xw_sb[p0 : p0 + K, :N_F],
                        start=True,
                        stop=True,
                    )
                ins.then_inc(s_mm, 1)

        @block.scalar
        def _(scalar):
            scalar.dma_start(
                out=xw_sb[:, SPLIT:], in_=xw_ext[:, SPLIT:]
            ).then_inc(s_in2, 16)
            for b in range(N_SB):
                scalar.wait_ge(s_mm, b + 1)
                nc.scalar.activation(
                    exp_slot(b), psum_banks[b % N_BANK][:], Exp
                ).then_inc(s_exp, 1)
            for bb in range(N_ACT_U2):
                if bb % RB == 0:
                    # first DVE mul of this batch implies its reciprocal
                    # completed (an ACT wait on s_rc faults at runtime)
                    scalar.wait_ge(s_mul, muls_before_batch[bb // RB] + 1)
                t2 = SB * bb + 2
                nc.scalar.activation(
                    sst[:, t2 * N_F : (t2 + 1) * N_F],
                    exp_slot(bb)[:, 2 * N_F : 3 * N_F],
                    Relu,  # x>=0 so Relu(x*scale) == x*scale
                    scale=recip_w[:, t2 : t2 + 1],
                ).then_inc(s_mula, 1)

        @block.vector
        def _(vector):
            for q in range(N_RB):
                b0 = RB * q
                vector.wait_ge(s_exp, b0 + RB)
                nc.vector.reduce_sum(
                    out=sums_w[:, SB * b0 : SB * (b0 + RB)],
                    in_=exp_buf[
                        :, b0 * W_SLOT : (b0 + RB) * W_SLOT
                    ].rearrange("p (t f) -> p t f", t=RB * SB),
                    axis=X,
                ).then_inc(s_red, 1)
                vector.wait_ge(s_red, q + 1)
                nc.vector.reciprocal(
                    recip_w[:, SB * b0 : SB * (b0 + RB)],
                    sums_w[:, SB * b0 : SB * (b0 + RB)],
                ).then_inc(s_rc, 1)
                vector.wait_ge(s_rc, q + 1)
                NTOT = N_TILES * N_F
                ETOT = N_SB * W_SLOT
                for half in range(2):
                    bp = b0 + 2 * half  # block pair (bp, bp+1), tiles u0/u1
                    t0 = SB * bp
                    rec_b = RawAP(
                        recip_w, t0,
                        [[N_TILES, 128], [SB, 2], [1, 2], [0, N_F]],
                    )
                    nc.vector.tensor_mul(
                        RawAP(
                            sst, t0 * N_F,
                            [[NTOT, 128], [W_SLOT, 2], [N_F, 2], [1, N_F]],
                        ),
                        RawAP(
                            exp_buf, bp * W_SLOT,
                            [[ETOT, 128], [W_SLOT, 2], [N_F, 2], [1, N_F]],
                        ),
                        rec_b,
                    ).then_inc(s_mul, 1)
                if b0 >= N_ACT_U2:
                    for half in range(2):
                        bp = b0 + 2 * half  # u2 tiles of (bp, bp+1)
                        t2 = SB * bp + 2
                        rec_b = RawAP(
                            recip_w, t2,
                            [[N_TILES, 128], [SB, 2], [0, N_F]],
                        )
                        nc.vector.tensor_mul(
                            RawAP(
                                sst, t2 * N_F,
                                [[NTOT, 128], [W_SLOT, 2], [1, N_F]],
                            ),
                            RawAP(
                                exp_buf, bp * W_SLOT + 2 * N_F,
                                [[ETOT, 128], [W_SLOT, 2], [1, N_F]],
                            ),
                            rec_b,
                        ).then_inc(s_mul, 1)

    nc.finalize()
    return nc


def _get_graph():
    global _GRAPH
    if _GRAPH is None:
        _GRAPH = _build_graph()
    return _GRAPH


def _prepare_inputs(inputs, W, b):
    """Host-side im2col + per-core packing (bf16)."""
    import ml_dtypes
    from numpy.lib.stride_tricks import sliding_window_view

    x = np.ascontiguousarray(np.asarray(inputs, dtype=np.float32)[0])  # [222,222,3]
    W = np.asarray(W, dtype=np.float32)
    b = np.asarray(b, dtype=np.float32)

    # [220,220,3(c),3(dy),3(dx)] -> [y,x,dy,dx,c] -> [48400, 27]
    win = sliding_window_view(x, (3, 3), axis=(0, 1))
    cols = win.transpose(0, 1, 3, 4, 2).reshape(N_PIX, 27)
    cols = np.concatenate(
        [cols, np.ones((N_PIX, 1), dtype=np.float32)], axis=1
    )  # [48400, 28]

    w28 = np.concatenate([W.reshape(27, N_F), b[None, :]], axis=0)  # [28,156]

    in_maps = []
    for i in range(N_CORES):
        shard = cols[i * PER_CORE : (i + 1) * PER_CORE]
        pad = np.zeros((PIX_PAD, K), dtype=np.float32)
        pad[:PER_CORE] = shard
        xw = np.zeros((128, XW_COLS), dtype=np.float32)
        for g in range(GROUPS):
            xw[32 * g : 32 * g + K, :N_F] = w28
            xw[32 * g : 32 * g + K, N_F:] = pad[g * G_PIX : (g + 1) * G_PIX].T
        in_maps.append({"xw": xw.astype(ml_dtypes.bfloat16)})
    return in_maps


def _run(inputs, W, b, trace=False):
    from concourse.bass_utils import run_bass_kernel_spmd

    in_maps = _prepare_inputs(inputs, W, b)
    nc = _get_graph()
    res = run_bass_kernel_spmd(
        nc, in_maps, core_ids=list(range(N_CORES)), trace=trace
    )

    S = np.empty((N_PIX, N_F), dtype=np.float32)
    for i in range(N_CORES):
        S[i * PER_CORE : (i + 1) * PER_CORE] = (
            res.results[i]["s"].reshape(PIX_PAD, N_F)[:PER_CORE]
        ).astype(np.float32)
    c = S.sum(axis=0, dtype=np.float64).astype(np.float32)
    new_adj = np.outer(c, c).astype(np.float32)
    return (new_adj, S), res


def kernel(**inputs):
    (new_adj, S), _ = _run(inputs["inputs"], inputs["W"], inputs["b"])
    return (new_adj, S)


# revision 32
# speedup vs baseline: 1.0782x; 1.0782x over previous
"""Distributed Trainium2 kernel for nn_AdjLayer (conv3x3 -> softmax -> outer(colsum)).

Raw-Bacc implementation (no TileContext) with a manual semaphore pipeline.

  - Host: im2col the tiny input (48400 x 28, incl. ones column for bias) in
    bf16, shard 6050 pixels per core, pad to 6144, pack as [128, 2204] with
    3 pixel-groups at partition offsets 0/32/64 (28 im2col rows each) and
    the weights in the first 156 columns.
  - Device (SPMD x8), per 3-tile superblock b (3 x 128 pixels):
      PE:   3 bf16 matmuls (K=28) -> PSUM bank b%8 [128, 468]
      ACT:  one wide Exp -> contiguous exp buffer slot b%8;
            normalize of tile u2 (Relu x scale == multiply, values >= 0)
      DVE:  batched 3D row-sum reduce (4 blocks at once); batched
            reciprocal; normalize of tiles u0,u1 (broadcast multiply)
      SYNC: S streamed out in 6 chunked DMAs (8 tiles each)
  - Host: gather S shards, c = S.sum(0), new_adj = outer(c, c).
"""

import sys
from contextlib import ExitStack

import numpy as np

for _p in ("/opt/trn_rl_repo",):
    if _p not in sys.path:
        sys.path.insert(0, _p)

N_F = 156          # filters
N_PIX = 48400      # 220*220 output pixels
N_CORES = 8
PER_CORE = N_PIX // N_CORES   # 6050
K = 28             # 27 conv taps + 1 ones-row (bias)
GROUPS = 3         # pixel groups at partition offsets 0/32/64
TILE = 128         # pixels per matmul tile
TILES_PER_G = 16
G_PIX = TILES_PER_G * TILE        # 2048 pixels per group
PIX_PAD = GROUPS * G_PIX          # 6144 padded pixels per core
N_TILES = GROUPS * TILES_PER_G    # 48
SB = 3                            # tiles per superblock
N_SB = N_TILES // SB              # 16
RB = 4                            # superblocks per reduce/reciprocal batch
CHUNK = 4                         # tiles per output DMA (12 DMAs)
N_CHUNKS = N_TILES // CHUNK
XW_COLS = N_F + G_PIX             # weights cols [0:156], im2col [156:2204]
SPLIT0 = N_F + 4 * TILE           # input dma0 covers W + tiles j<4
SPLIT = N_F + 12 * TILE           # input dma1 covers tiles j<12
N_SLOT = 8                        # psum banks in flight (exp slots = N_SB)

_GRAPH = None


def _build_graph():
    from concourse import bacc
    from concourse import mybir

    f32 = mybir.dt.float32
    bf16 = mybir.dt.bfloat16
    nc = bacc.Bacc(None)

    xw_ext = nc.declare_dram_parameter("xw", [128, XW_COLS], bf16, isOutput=False)
    # [48, 128, 156] is byte-identical to [6144, 156] (tile-major rows)
    s_ext = nc.declare_dram_parameter("s", [N_TILES, TILE, N_F], bf16, isOutput=True)

    Exp = mybir.ActivationFunctionType.Exp
    Relu = mybir.ActivationFunctionType.Relu
    X = mybir.AxisListType.X

    W_SLOT = SB * N_F  # 468 columns per exp-buffer slot

    # completion bookkeeping: after DVE mul of block b, s_mul == b+1;
    # after ACT relu-scale of block b, s_mula == b+1.  Chunk c needs
    # every tile <= L staged: tiles 3b,3b+1 by DVE, 3b+2 by ACT.
    DVE_THR = []
    ACT_THR = []
    for c in range(N_CHUNKS):
        L = CHUNK * c + CHUNK - 1
        DVE_THR.append(L // SB + 1)
        ACT_THR.append((L - 2) // SB + 1)

    with ExitStack() as ctx:
        xw_sb = ctx.enter_context(nc.sbuf_tensor("xw_sb", [128, XW_COLS], bf16))
        sst = ctx.enter_context(nc.sbuf_tensor("sst", [128, N_TILES * N_F], bf16))
        sums_w = ctx.enter_context(nc.sbuf_tensor("sums_w", [128, N_TILES], f32))
        recip_w = ctx.enter_context(nc.sbuf_tensor("recip_w", [128, N_TILES], f32))
        exp_buf = ctx.enter_context(
            nc.sbuf_tensor("exp_buf", [128, N_SB * W_SLOT], f32)
        )
        psum_banks = [
            ctx.enter_context(nc.psum_tensor(f"psbank{i}", [128, W_SLOT], f32))
            for i in range(N_SLOT)
        ]
        s_in = ctx.enter_context(nc.semaphore("s_in"))
        s_in0 = ctx.enter_context(nc.semaphore("s_in0"))
        s_in2 = ctx.enter_context(nc.semaphore("s_in2"))
        s_mm = ctx.enter_context(nc.semaphore("s_mm"))
        s_exp = ctx.enter_context(nc.semaphore("s_exp"))
        s_red = ctx.enter_context(nc.semaphore("s_red"))
        s_rc = ctx.enter_context(nc.semaphore("s_rc"))
        s_mul = ctx.enter_context(nc.semaphore("s_mul"))
        s_mula = ctx.enter_context(nc.semaphore("s_mula"))
        s_out = ctx.enter_context(nc.semaphore("s_out"))

        block = ctx.enter_context(nc.Block(no_gpsimd_drain=True))

        def exp_slot(b):
            s0 = b * W_SLOT
            return exp_buf[:, s0 : s0 + W_SLOT]

        @block.sync
        def _(sync):
            sync.dma_start(
                out=xw_sb[:, :SPLIT0], in_=xw_ext[:, :SPLIT0]
            ).then_inc(s_in0, 16)
            sync.dma_start(
                out=xw_sb[:, SPLIT0:SPLIT], in_=xw_ext[:, SPLIT0:SPLIT]
            ).then_inc(s_in, 16)
            for c in range(N_CHUNKS):
                sync.wait_ge(s_mul, DVE_THR[c])
                sync.wait_ge(s_mula, ACT_THR[c])
                t0 = c * CHUNK
                dst = s_ext[t0 : t0 + CHUNK].rearrange("t p f -> p t f")
                src = sst[:, t0 * N_F : (t0 + CHUNK) * N_F].rearrange(
                    "p (t f) -> p t f", t=CHUNK
                )
                sync.dma_start(out=dst, in_=src).then_inc(s_out, 16)
            sync.wait_ge(s_out, 16 * N_CHUNKS)

        @block.tensor
        def _(tensor):
            tensor.wait_ge(s_in0, 16)
            for b in range(N_SB):
                if b == 1:
                    tensor.wait_ge(s_in, 16)
                if b == 4:
                    tensor.wait_ge(s_in2, 16)
                if b >= N_SLOT:
                    tensor.wait_ge(s_exp, b - N_SLOT + 1)
                pb = psum_banks[b % N_SLOT]
                for u in range(SB):
                    t = SB * b + u
                    g, j = divmod(t, TILES_PER_G)
                    p0 = 32 * g
                    ins = nc.tensor.matmul(
                        pb[:, u * N_F : (u + 1) * N_F],
                        lhsT=xw_sb[
                            p0 : p0 + K, N_F + j * TILE : N_F + (j + 1) * TILE
                        ],
                        rhs=xw_sb[p0 : p0 + K, :N_F],
                        start=True,
                        stop=True,
                    )
                ins.then_inc(s_mm, 1)

        @block.scalar
        def _(scalar):
            scalar.dma_start(
                out=xw_sb[:, SPLIT:], in_=xw_ext[:, SPLIT:]
            ).then_inc(s_in2, 16)
            for b in range(N_SB):
                scalar.wait_ge(s_mm, b + 1)
                nc.scalar.activation(
                    exp_slot(b), psum_banks[b % N_SLOT][:], Exp
                ).then_inc(s_exp, 1)
            for bb in range(N_SB):
                if bb % RB == 0:
                    # first DVE mul of this batch implies the reciprocal
                    # completed (ACT->s_rc waits fault at runtime; s_mul works)
                    scalar.wait_ge(s_mul, bb + 1)
                t2 = SB * bb + 2
                nc.scalar.activation(
                    sst[:, t2 * N_F : (t2 + 1) * N_F],
                    exp_slot(bb)[:, 2 * N_F : 3 * N_F],
                    Relu,  # x>=0 so Relu(x*scale) == x*scale
                    scale=recip_w[:, t2 : t2 + 1],
                ).then_inc(s_mula, 1)

        @block.vector
        def _(vector):
            for b in range(N_SB):
                vector.wait_ge(s_exp, b + 1)
                nc.vector.reduce_sum(
                    out=sums_w[:, SB * b : SB * (b + 1)],
                    in_=exp_slot(b).rearrange("p (t f) -> p t f", t=SB),
                    axis=X,
                ).then_inc(s_red, 1)
                if b % RB != RB - 1:
                    continue
                q = b // RB
                vector.wait_ge(s_red, b + 1)
                nc.vector.reciprocal(
                    recip_w[:, SB * (b - RB + 1) : SB * (b + 1)],
                    sums_w[:, SB * (b - RB + 1) : SB * (b + 1)],
                ).then_inc(s_rc, 1)
                vector.wait_ge(s_rc, q + 1)
                for bb in range(b - RB + 1, b + 1):
                    t0 = SB * bb
                    rec_b = (
                        recip_w[:, t0 : t0 + 2]
                        .rearrange("p (t o) -> p t o", o=1)
                        .broadcast_to([128, 2, N_F])
                    )
                    nc.vector.tensor_mul(
                        sst[:, t0 * N_F : (t0 + 2) * N_F].rearrange(
                            "p (t f) -> p t f", t=2
                        ),
                        exp_slot(bb)[:, : 2 * N_F].rearrange(
                            "p (t f) -> p t f", t=2
                        ),
                        rec_b,
                    ).then_inc(s_mul, 1)

    nc.finalize()
    return nc


def _get_graph():
    global _GRAPH
    if _GRAPH is None:
        _GRAPH = _build_graph()
    return _GRAPH


def _prepare_inputs(inputs, W, b):
    """Host-side im2col + per-core packing (bf16)."""
    import ml_dtypes
    from numpy.lib.stride_tricks import sliding_window_view

    x = np.ascontiguousarray(np.asarray(inputs, dtype=np.float32)[0])  # [222,222,3]
    W = np.asarray(W, dtype=np.float32)
    b = np.asarray(b, dtype=np.float32)

    # [220,220,3(c),3(dy),3(dx)] -> [y,x,dy,dx,c] -> [48400, 27]
    win = sliding_window_view(x, (3, 3), axis=(0, 1))
    cols = win.transpose(0, 1, 3, 4, 2).reshape(N_PIX, 27)
    cols = np.concatenate(
        [cols, np.ones((N_PIX, 1), dtype=np.float32)], axis=1
    )  # [48400, 28]

    w28 = np.concatenate([W.reshape(27, N_F), b[None, :]], axis=0)  # [28,156]

    in_maps = []
    for i in range(N_CORES):
        shard = cols[i * PER_CORE : (i + 1) * PER_CORE]
        pad = np.zeros((PIX_PAD, K), dtype=np.float32)
        pad[:PER_CORE] = shard
        xw = np.zeros((128, XW_COLS), dtype=np.float32)
        for g in range(GROUPS):
            xw[32 * g : 32 * g + K, :N_F] = w28
            xw[32 * g : 32 * g + K, N_F:] = pad[g * G_PIX : (g + 1) * G_PIX].T
        in_maps.append({"xw": xw.astype(ml_dtypes.bfloat16)})
    return in_maps


def _run(inputs, W, b, trace=False):
    from concourse.bass_utils import run_bass_kernel_spmd

    in_maps = _prepare_inputs(inputs, W, b)
    nc = _get_graph()
    res = run_bass_kernel_spmd(
        nc, in_maps, core_ids=list(range(N_CORES)), trace=trace
    )

    S = np.empty((N_PIX, N_F), dtype=np.float32)
    for i in range(N_CORES):
        S[i * PER_CORE : (i + 1) * PER_CORE] = (
            res.results[i]["s"].reshape(PIX_PAD, N_F)[:PER_CORE]
        ).astype(np.float32)
    c = S.sum(axis=0, dtype=np.float64).astype(np.float32)
    new_adj = np.outer(c, c).astype(np.float32)
    return (new_adj, S), res


def kernel(**inputs):
    (new_adj, S), _ = _run(inputs["inputs"], inputs["W"], inputs["b"])
    return (new_adj, S)


# revision 34
# speedup vs baseline: 1.3209x; 1.2251x over previous
"""Distributed Trainium2 kernel for nn_AdjLayer (conv3x3 -> softmax -> outer(colsum)).

Raw-Bacc implementation (no TileContext) with a manual semaphore pipeline.

  - Host: im2col the tiny input (48400 x 28, incl. ones column for bias) in
    bf16, shard 6050 pixels per core, pad to 6144, pack as [128, 2204] with
    3 pixel-groups at partition offsets 0/32/64 (28 im2col rows each) and
    the weights in the first 156 columns.
  - Device (SPMD x8), per 3-tile superblock b (3 x 128 pixels):
      PE:   3 bf16 matmuls (K=28) -> PSUM bank b%8 [128, 468]
      ACT:  one wide Exp -> contiguous exp buffer slot b%8;
            normalize of tile u2 (Relu x scale == multiply, values >= 0)
      DVE:  batched 3D row-sum reduce (4 blocks at once); batched
            reciprocal; normalize of tiles u0,u1 (broadcast multiply)
      SYNC: S streamed out in 6 chunked DMAs (8 tiles each)
  - Host: gather S shards, c = S.sum(0), new_adj = outer(c, c).
"""

import sys
from contextlib import ExitStack

import numpy as np

for _p in ("/opt/trn_rl_repo",):
    if _p not in sys.path:
        sys.path.insert(0, _p)

N_F = 156          # filters
N_PIX = 48400      # 220*220 output pixels
N_CORES = 8
PER_CORE = N_PIX // N_CORES   # 6050
K = 28             # 27 conv taps + 1 ones-row (bias)
GROUPS = 3         # pixel groups at partition offsets 0/32/64
TILE = 128         # pixels per matmul tile
TILES_PER_G = 16
G_PIX = TILES_PER_G * TILE        # 2048 pixels per group
PIX_PAD = GROUPS * G_PIX          # 6144 padded pixels per core
N_TILES = GROUPS * TILES_PER_G    # 48
SB = 3                            # tiles per superblock
N_SB = N_TILES // SB              # 16
RB = 4                            # superblocks per reduce/reciprocal batch
CHUNK = 4                         # tiles per output DMA (12 DMAs)
N_CHUNKS = N_TILES // CHUNK
XW_COLS = N_F + G_PIX             # weights cols [0:156], im2col [156:2204]
SPLIT0 = N_F + 4 * TILE           # input dma0 covers W + tiles j<4
SPLIT = N_F + 12 * TILE           # input dma1 covers tiles j<12
N_SLOT = 8                        # psum banks in flight (exp slots = N_SB)

_GRAPH = None


def _build_graph():
    from concourse import bacc
    from concourse import mybir
    from concourse.ap import AP as RawAP

    f32 = mybir.dt.float32
    bf16 = mybir.dt.bfloat16
    nc = bacc.Bacc(None)

    xw_ext = nc.declare_dram_parameter("xw", [128, XW_COLS], bf16, isOutput=False)
    # [48, 128, 156] is byte-identical to [6144, 156] (tile-major rows)
    s_ext = nc.declare_dram_parameter("s", [N_TILES, TILE, N_F], f32, isOutput=True)

    Exp = mybir.ActivationFunctionType.Exp
    Relu = mybir.ActivationFunctionType.Relu
    X = mybir.AxisListType.X

    W_SLOT = SB * N_F  # 468 columns per exp-buffer slot

    # completion bookkeeping: after DVE mul of block b, s_mul == b+1;
    # after ACT relu-scale of block b, s_mula == b+1.  Chunk c needs
    # every tile <= L staged: tiles 3b,3b+1 by DVE, 3b+2 by ACT.
    DVE_THR = []
    ACT_THR = []
    for c in range(N_CHUNKS):
        L = CHUNK * c + CHUNK - 1
        DVE_THR.append(L // SB + 1)
        ACT_THR.append((L - 2) // SB + 1)

    with ExitStack() as ctx:
        xw_sb = ctx.enter_context(nc.sbuf_tensor("xw_sb", [128, XW_COLS], bf16))
        sst = ctx.enter_context(nc.sbuf_tensor("sst", [128, N_TILES * N_F], f32))
        sums_w = ctx.enter_context(nc.sbuf_tensor("sums_w", [128, N_TILES], f32))
        recip_w = ctx.enter_context(nc.sbuf_tensor("recip_w", [128, N_TILES], f32))
        exp_buf = ctx.enter_context(
            nc.sbuf_tensor("exp_buf", [128, N_SB * W_SLOT], f32)
        )
        psum_banks = [
            ctx.enter_context(nc.psum_tensor(f"psbank{i}", [128, W_SLOT], f32))
            for i in range(N_SLOT)
        ]
        s_in = ctx.enter_context(nc.semaphore("s_in"))
        s_in0 = ctx.enter_context(nc.semaphore("s_in0"))
        s_in2 = ctx.enter_context(nc.semaphore("s_in2"))
        s_mm = ctx.enter_context(nc.semaphore("s_mm"))
        s_exp = ctx.enter_context(nc.semaphore("s_exp"))
        s_red = ctx.enter_context(nc.semaphore("s_red"))
        s_rc = ctx.enter_context(nc.semaphore("s_rc"))
        s_mul = ctx.enter_context(nc.semaphore("s_mul"))
        s_mula = ctx.enter_context(nc.semaphore("s_mula"))
        s_out = ctx.enter_context(nc.semaphore("s_out"))

        block = ctx.enter_context(nc.Block(no_gpsimd_drain=True))

        def exp_slot(b):
            s0 = b * W_SLOT
            return exp_buf[:, s0 : s0 + W_SLOT]

        @block.sync
        def _(sync):
            sync.dma_start(
                out=xw_sb[:, :SPLIT0], in_=xw_ext[:, :SPLIT0]
            ).then_inc(s_in0, 16)
            sync.dma_start(
                out=xw_sb[:, SPLIT0:SPLIT], in_=xw_ext[:, SPLIT0:SPLIT]
            ).then_inc(s_in, 16)
            for c in range(N_CHUNKS):
                sync.wait_ge(s_mul, DVE_THR[c])
                sync.wait_ge(s_mula, ACT_THR[c])
                t0 = c * CHUNK
                # pixel(tile 4m+e, part p) == 512m + 4p + e, so one
                # chunk is a contiguous [128, 624] block of DRAM rows
                dst = RawAP(
                    s_ext,
                    t0 * TILE * N_F,
                    [[CHUNK * N_F, 128], [1, CHUNK * N_F]],
                )
                src = sst[:, t0 * N_F : (t0 + CHUNK) * N_F]
                sync.dma_start(out=dst, in_=src).then_inc(s_out, 16)
            sync.wait_ge(s_out, 16 * N_CHUNKS)

        @block.tensor
        def _(tensor):
            tensor.wait_ge(s_in0, 16)
            for b in range(N_SB):
                if b == 1:
                    tensor.wait_ge(s_in, 16)
                if b == 4:
                    tensor.wait_ge(s_in2, 16)
                if b >= N_SLOT:
                    tensor.wait_ge(s_exp, b - N_SLOT + 1)
                pb = psum_banks[b % N_SLOT]
                for u in range(SB):
                    t = SB * b + u
                    g, j = divmod(t, TILES_PER_G)
                    p0 = 32 * g
                    ins = nc.tensor.matmul(
                        pb[:, u * N_F : (u + 1) * N_F],
                        lhsT=xw_sb[
                            p0 : p0 + K, N_F + j * TILE : N_F + (j + 1) * TILE
                        ],
                        rhs=xw_sb[p0 : p0 + K, :N_F],
                        start=True,
                        stop=True,
                    )
                ins.then_inc(s_mm, 1)

        @block.scalar
        def _(scalar):
            scalar.dma_start(
                out=xw_sb[:, SPLIT:], in_=xw_ext[:, SPLIT:]
            ).then_inc(s_in2, 16)
            for b in range(N_SB):
                scalar.wait_ge(s_mm, b + 1)
                nc.scalar.activation(
                    exp_slot(b), psum_banks[b % N_SLOT][:], Exp
                ).then_inc(s_exp, 1)
            for bb in range(N_SB):
                if bb % RB == 0:
                    # first DVE mul of this batch implies the reciprocal
                    # completed (ACT->s_rc waits fault at runtime; s_mul works)
                    scalar.wait_ge(s_mul, bb + 1)
                t2 = SB * bb + 2
                nc.scalar.activation(
                    sst[:, t2 * N_F : (t2 + 1) * N_F],
                    exp_slot(bb)[:, 2 * N_F : 3 * N_F],
                    Relu,  # x>=0 so Relu(x*scale) == x*scale
                    scale=recip_w[:, t2 : t2 + 1],
                ).then_inc(s_mula, 1)

        @block.vector
        def _(vector):
            for b in range(N_SB):
                vector.wait_ge(s_exp, b + 1)
                nc.vector.reduce_sum(
                    out=sums_w[:, SB * b : SB * (b + 1)],
                    in_=exp_slot(b).rearrange("p (t f) -> p t f", t=SB),
                    axis=X,
                ).then_inc(s_red, 1)
                if b % RB != RB - 1:
                    continue
                q = b // RB
                vector.wait_ge(s_red, b + 1)
                nc.vector.reciprocal(
                    recip_w[:, SB * (b - RB + 1) : SB * (b + 1)],
                    sums_w[:, SB * (b - RB + 1) : SB * (b + 1)],
                ).then_inc(s_rc, 1)
                vector.wait_ge(s_rc, q + 1)
                for bb in range(b - RB + 1, b + 1):
                    t0 = SB * bb
                    rec_b = (
                        recip_w[:, t0 : t0 + 2]
                        .rearrange("p (t o) -> p t o", o=1)
                        .broadcast_to([128, 2, N_F])
                    )
                    nc.vector.tensor_mul(
                        sst[:, t0 * N_F : (t0 + 2) * N_F].rearrange(
                            "p (t f) -> p t f", t=2
                        ),
                        exp_slot(bb)[:, : 2 * N_F].rearrange(
                            "p (t f) -> p t f", t=2
                        ),
                        rec_b,
                    ).then_inc(s_mul, 1)

    nc.finalize()
    return nc


def _get_graph():
    global _GRAPH
    if _GRAPH is None:
        _GRAPH = _build_graph()
    return _GRAPH


def _prepare_inputs(inputs, W, b):
    """Host-side im2col + per-core packing (bf16)."""
    import ml_dtypes
    from numpy.lib.stride_tricks import sliding_window_view

    x = np.ascontiguousarray(np.asarray(inputs, dtype=np.float32)[0])  # [222,222,3]
    W = np.asarray(W, dtype=np.float32)
    b = np.asarray(b, dtype=np.float32)

    # [220,220,3(c),3(dy),3(dx)] -> [y,x,dy,dx,c] -> [48400, 27]
    win = sliding_window_view(x, (3, 3), axis=(0, 1))
    cols = win.transpose(0, 1, 3, 4, 2).reshape(N_PIX, 27)
    cols = np.concatenate(
        [cols, np.ones((N_PIX, 1), dtype=np.float32)], axis=1
    )  # [48400, 28]

    w28 = np.concatenate([W.reshape(27, N_F), b[None, :]], axis=0)  # [28,156]

    in_maps = []
    for i in range(N_CORES):
        shard = cols[i * PER_CORE : (i + 1) * PER_CORE]
        pad = np.zeros((PIX_PAD, K), dtype=np.float32)
        pad[:PER_CORE] = shard
        # tile 4m+e, column j computes core-pixel 512m + 4j + e
        perm = (
            pad.reshape(12, TILE, 4, K).transpose(0, 2, 1, 3).reshape(N_TILES, TILE, K)
        )
        xw = np.zeros((128, XW_COLS), dtype=np.float32)
        for g in range(GROUPS):
            xw[32 * g : 32 * g + K, :N_F] = w28
            xw[32 * g : 32 * g + K, N_F:] = (
                perm[TILES_PER_G * g : TILES_PER_G * (g + 1)]
                .reshape(G_PIX, K)
                .T
            )
        in_maps.append({"xw": xw.astype(ml_dtypes.bfloat16)})
    return in_maps


def _run(inputs, W, b, trace=False):
    from concourse.bass_utils import run_bass_kernel_spmd

    in_maps = _prepare_inputs(inputs, W, b)
    nc = _get_graph()
    res = run_bass_kernel_spmd(
        nc, in_maps, core_ids=list(range(N_CORES)), trace=trace
    )

    S = np.empty((N_PIX, N_F), dtype=np.float32)
    for i in range(N_CORES):
        # quad-interleaved staging lands in natural pixel order in DRAM
        S[i * PER_CORE : (i + 1) * PER_CORE] = res.results[i]["s"].reshape(
            PIX_PAD, N_F
        )[:PER_CORE]
    c = S.sum(axis=0, dtype=np.float64).astype(np.float32)
    new_adj = np.outer(c, c).astype(np.float32)
    return (new_adj, S), res


def kernel(**inputs):
    (new_adj, S), _ = _run(inputs["inputs"], inputs["W"], inputs["b"])
    return (new_adj, S)


# revision 35
# speedup vs baseline: 1.3354x; 1.0110x over previous
"""Distributed Trainium2 kernel for nn_AdjLayer (conv3x3 -> softmax -> outer(colsum)).

Raw-Bacc implementation (no TileContext) with a manual semaphore pipeline.

  - Host: im2col the tiny input (48400 x 28, incl. ones column for bias) in
    bf16, shard 6050 pixels per core, pad to 6144, pack as [128, 2204] with
    3 pixel-groups at partition offsets 0/32/64 (28 im2col rows each) and
    the weights in the first 156 columns.
  - Device (SPMD x8), per 3-tile superblock b (3 x 128 pixels):
      PE:   3 bf16 matmuls (K=28) -> PSUM bank b%8 [128, 468]
      ACT:  one wide Exp -> contiguous exp buffer slot b%8;
            normalize of tile u2 (Relu x scale == multiply, values >= 0)
      DVE:  batched 3D row-sum reduce (4 blocks at once); batched
            reciprocal; normalize of tiles u0,u1 (broadcast multiply)
      SYNC: S streamed out in 6 chunked DMAs (8 tiles each)
  - Host: gather S shards, c = S.sum(0), new_adj = outer(c, c).
"""

import sys
from contextlib import ExitStack

import numpy as np

for _p in ("/opt/trn_rl_repo",):
    if _p not in sys.path:
        sys.path.insert(0, _p)

N_F = 156          # filters
N_PIX = 48400      # 220*220 output pixels
N_CORES = 8
PER_CORE = N_PIX // N_CORES   # 6050
K = 28             # 27 conv taps + 1 ones-row (bias)
GROUPS = 3         # pixel groups at partition offsets 0/32/64
TILE = 128         # pixels per matmul tile
TILES_PER_G = 16
G_PIX = TILES_PER_G * TILE        # 2048 pixels per group
PIX_PAD = GROUPS * G_PIX          # 6144 padded pixels per core
N_TILES = GROUPS * TILES_PER_G    # 48
SB = 3                            # tiles per superblock
N_SB = N_TILES // SB              # 16
RB = 4                            # superblocks per reduce/reciprocal batch
CHUNK = 4                         # tiles per output DMA (12 DMAs)
N_CHUNKS = N_TILES // CHUNK
XW_COLS = N_F + G_PIX             # weights cols [0:156], im2col [156:2204]
SPLIT0 = N_F + 4 * TILE           # input dma0 covers W + tiles j<4
SPLIT = N_F + 12 * TILE           # input dma1 covers tiles j<12
N_SLOT = 8                        # psum banks in flight (exp slots = N_SB)

_GRAPH = None


def _build_graph():
    from concourse import bacc
    from concourse import mybir
    from concourse.ap import AP as RawAP

    f32 = mybir.dt.float32
    bf16 = mybir.dt.bfloat16
    nc = bacc.Bacc(None)

    xw_ext = nc.declare_dram_parameter("xw", [128, XW_COLS], bf16, isOutput=False)
    # [48, 128, 156] is byte-identical to [6144, 156] (tile-major rows)
    s_ext = nc.declare_dram_parameter("s", [N_TILES, TILE, N_F], f32, isOutput=True)

    Exp = mybir.ActivationFunctionType.Exp
    Relu = mybir.ActivationFunctionType.Relu
    X = mybir.AxisListType.X

    W_SLOT = SB * N_F  # 468 columns per exp-buffer slot

    # completion bookkeeping: after DVE mul of block b, s_mul == b+1;
    # after ACT relu-scale of block b, s_mula == b+1.  Chunk c needs
    # every tile <= L staged: tiles 3b,3b+1 by DVE, 3b+2 by ACT.
    DVE_THR = []
    ACT_THR = []
    for c in range(N_CHUNKS):
        L = CHUNK * c + CHUNK - 1
        DVE_THR.append(L // SB + 1)
        ACT_THR.append((L - 2) // SB + 1)

    with ExitStack() as ctx:
        xw_sb = ctx.enter_context(nc.sbuf_tensor("xw_sb", [128, XW_COLS], bf16))
        sst = ctx.enter_context(nc.sbuf_tensor("sst", [128, N_TILES * N_F], f32))
        sums_w = ctx.enter_context(nc.sbuf_tensor("sums_w", [128, N_TILES], f32))
        recip_w = ctx.enter_context(nc.sbuf_tensor("recip_w", [128, N_TILES], f32))
        exp_buf = ctx.enter_context(
            nc.sbuf_tensor("exp_buf", [128, N_SB * W_SLOT], f32)
        )
        psum_banks = [
            ctx.enter_context(nc.psum_tensor(f"psbank{i}", [128, W_SLOT], f32))
            for i in range(N_SLOT)
        ]
        s_in = ctx.enter_context(nc.semaphore("s_in"))
        s_in0 = ctx.enter_context(nc.semaphore("s_in0"))
        s_in2 = ctx.enter_context(nc.semaphore("s_in2"))
        s_mm = ctx.enter_context(nc.semaphore("s_mm"))
        s_exp = ctx.enter_context(nc.semaphore("s_exp"))
        s_red = ctx.enter_context(nc.semaphore("s_red"))
        s_rc = ctx.enter_context(nc.semaphore("s_rc"))
        s_mul = ctx.enter_context(nc.semaphore("s_mul"))
        s_mula = ctx.enter_context(nc.semaphore("s_mula"))
        s_out = ctx.enter_context(nc.semaphore("s_out"))

        block = ctx.enter_context(nc.Block(no_gpsimd_drain=True))

        def exp_slot(b):
            s0 = b * W_SLOT
            return exp_buf[:, s0 : s0 + W_SLOT]

        @block.sync
        def _(sync):
            sync.dma_start(
                out=xw_sb[:, :SPLIT0], in_=xw_ext[:, :SPLIT0]
            ).then_inc(s_in0, 16)
            sync.dma_start(
                out=xw_sb[:, SPLIT0:SPLIT], in_=xw_ext[:, SPLIT0:SPLIT]
            ).then_inc(s_in, 16)
            for c in range(N_CHUNKS):
                sync.wait_ge(s_mul, DVE_THR[c])
                sync.wait_ge(s_mula, ACT_THR[c])
                t0 = c * CHUNK
                # pixel(tile 4m+e, part p) == 512m + 4p + e, so one
                # chunk is a contiguous [128, 624] block of DRAM rows
                dst = RawAP(
                    s_ext,
                    t0 * TILE * N_F,
                    [[CHUNK * N_F, 128], [1, CHUNK * N_F]],
                )
                src = sst[:, t0 * N_F : (t0 + CHUNK) * N_F]
                sync.dma_start(out=dst, in_=src).then_inc(s_out, 16)
            sync.wait_ge(s_out, 16 * N_CHUNKS)

        @block.tensor
        def _(tensor):
            tensor.wait_ge(s_in0, 16)
            for b in range(N_SB):
                if b == 1:
                    tensor.wait_ge(s_in, 16)
                if b == 4:
                    tensor.wait_ge(s_in2, 16)
                if b >= N_SLOT:
                    tensor.wait_ge(s_exp, b - N_SLOT + 1)
                pb = psum_banks[b % N_SLOT]
                for u in range(SB):
                    t = SB * b + u
                    g, j = divmod(t, TILES_PER_G)
                    p0 = 32 * g
                    ins = nc.tensor.matmul(
                        pb[:, u * N_F : (u + 1) * N_F],
                        lhsT=xw_sb[
                            p0 : p0 + K, N_F + j * TILE : N_F + (j + 1) * TILE
                        ],
                        rhs=xw_sb[p0 : p0 + K, :N_F],
                        start=True,
                        stop=True,
                    )
                ins.then_inc(s_mm, 1)

        @block.scalar
        def _(scalar):
            scalar.dma_start(
                out=xw_sb[:, SPLIT:], in_=xw_ext[:, SPLIT:]
            ).then_inc(s_in2, 16)
            def emit_exp(b):
                scalar.wait_ge(s_mm, b + 1)
                nc.scalar.activation(
                    exp_slot(b), psum_banks[b % N_SLOT][:], Exp
                ).then_inc(s_exp, 1)

            def emit_relu_batch(q):
                # first DVE mul of this batch implies the reciprocal
                # completed (ACT->s_rc waits fault at runtime; s_mul works)
                scalar.wait_ge(s_mul, RB * q + 1)
                for bb in range(RB * q, RB * (q + 1)):
                    t2 = SB * bb + 2
                    nc.scalar.activation(
                        sst[:, t2 * N_F : (t2 + 1) * N_F],
                        exp_slot(bb)[:, 2 * N_F : 3 * N_F],
                        Relu,  # x>=0 so Relu(x*scale) == x*scale
                        scale=recip_w[:, t2 : t2 + 1],
                    ).then_inc(s_mula, 1)

            # interleave so ACT stays dense: relu batch q slots in only
            # after its gating mul is comfortably done
            for b in range(8):
                emit_exp(b)
            emit_relu_batch(0)
            for b in range(8, 12):
                emit_exp(b)
            emit_relu_batch(1)
            for b in range(12, 16):
                emit_exp(b)
            emit_relu_batch(2)
            emit_relu_batch(3)

        @block.vector
        def _(vector):
            for b in range(N_SB):
                vector.wait_ge(s_exp, b + 1)
                nc.vector.reduce_sum(
                    out=sums_w[:, SB * b : SB * (b + 1)],
                    in_=exp_slot(b).rearrange("p (t f) -> p t f", t=SB),
                    axis=X,
                ).then_inc(s_red, 1)
                if b % RB != RB - 1:
                    continue
                q = b // RB
                vector.wait_ge(s_red, b + 1)
                nc.vector.reciprocal(
                    recip_w[:, SB * (b - RB + 1) : SB * (b + 1)],
                    sums_w[:, SB * (b - RB + 1) : SB * (b + 1)],
                ).then_inc(s_rc, 1)
                vector.wait_ge(s_rc, q + 1)
                for bb in range(b - RB + 1, b + 1):
                    t0 = SB * bb
                    rec_b = (
                        recip_w[:, t0 : t0 + 2]
                        .rearrange("p (t o) -> p t o", o=1)
                        .broadcast_to([128, 2, N_F])
                    )
                    nc.vector.tensor_mul(
                        sst[:, t0 * N_F : (t0 + 2) * N_F].rearrange(
                            "p (t f) -> p t f", t=2
                        ),
                        exp_slot(bb)[:, : 2 * N_F].rearrange(
                            "p (t f) -> p t f", t=2
                        ),
                        rec_b,
                    ).then_inc(s_mul, 1)

    nc.finalize()
    return nc


def _get_graph():
    global _GRAPH
    if _GRAPH is None:
        _GRAPH = _build_graph()
    return _GRAPH


def _prepare_inputs(inputs, W, b):
    """Host-side im2col + per-core packing (bf16)."""
    import ml_dtypes
    from numpy.lib.stride_tricks import sliding_window_view

    x = np.ascontiguousarray(np.asarray(inputs, dtype=np.float32)[0])  # [222,222,3]
    W = np.asarray(W, dtype=np.float32)
    b = np.asarray(b, dtype=np.float32)

    # [220,220,3(c),3(dy),3(dx)] -> [y,x,dy,dx,c] -> [48400, 27]
    win = sliding_window_view(x, (3, 3), axis=(0, 1))
    cols = win.transpose(0, 1, 3, 4, 2).reshape(N_PIX, 27)
    cols = np.concatenate(
        [cols, np.ones((N_PIX, 1), dtype=np.float32)], axis=1
    )  # [48400, 28]

    w28 = np.concatenate([W.reshape(27, N_F), b[None, :]], axis=0)  # [28,156]

    in_maps = []
    for i in range(N_CORES):
        shard = cols[i * PER_CORE : (i + 1) * PER_CORE]
        pad = np.zeros((PIX_PAD, K), dtype=np.float32)
        pad[:PER_CORE] = shard
        # tile 4m+e, column j computes core-pixel 512m + 4j + e
        perm = (
            pad.reshape(12, TILE, 4, K).transpose(0, 2, 1, 3).reshape(N_TILES, TILE, K)
        )
        xw = np.zeros((128, XW_COLS), dtype=np.float32)
        for g in range(GROUPS):
            xw[32 * g : 32 * g + K, :N_F] = w28
            xw[32 * g : 32 * g + K, N_F:] = (
                perm[TILES_PER_G * g : TILES_PER_G * (g + 1)]
                .reshape(G_PIX, K)
                .T
            )
        in_maps.append({"xw": xw.astype(ml_dtypes.bfloat16)})
    return in_maps


def _run(inputs, W, b, trace=False):
    from concourse.bass_utils import run_bass_kernel_spmd

    in_maps = _prepare_inputs(inputs, W, b)
    nc = _get_graph()
    res = run_bass_kernel_spmd(
        nc, in_maps, core_ids=list(range(N_CORES)), trace=trace
    )

    S = np.empty((N_PIX, N_F), dtype=np.float32)
    for i in range(N_CORES):
        # quad-interleaved staging lands in natural pixel order in DRAM
        S[i * PER_CORE : (i + 1) * PER_CORE] = res.results[i]["s"].reshape(
            PIX_PAD, N_F
        )[:PER_CORE]
    c = S.sum(axis=0, dtype=np.float64).astype(np.float32)
    new_adj = np.outer(c, c).astype(np.float32)
    return (new_adj, S), res


def kernel(**inputs):
    (new_adj, S), _ = _run(inputs["inputs"], inputs["W"], inputs["b"])
    return (new_adj, S)
